# revision 26
# baseline (speedup 1.0000x reference)
"""Trainium2 Bass kernel for nn_Encoding_74371653698241 (vq_codebook).

Reference (b=4, C=256, h=w=64, K=16):
    cw   = einsum('kc,bchw->bkhw', conv_w, x); BatchNorm(train) over (b,h,w); ReLU
    scale = mean_hw(cw)                     # (b,K)
    r    = cw.reshape(b, hw, K)             # flat reinterpretation
    diff[b,i,k,c] = x2[b,i,c] - r[b,i,k]
    sl = scale * sum_i diff^2; aw = softmax_k(sl); out = aw * sum_i diff

Algebraic reformulation (the (b,hw,K,C) diff tensor is never materialized):
    sum_i diff^2 = S2x[c] - 2*M[k,c] + SR2[k],   M = r^T @ x2
    sum_i diff   = Sx[c] - Sr[k]
M, Sr, SR2, Sx come out of ONE accumulated PE matmul chain
    [R | R^2 | 1]^T @ [x_t | 1 | 1]   (contraction over hw, 32 chunks of 128)
Sharding: core p handles batch p//2 (pairs redundant); the only cross-core
data is the BatchNorm mean/var -> one 8-rank AllGather of 32 floats.
"""

import sys
import numpy as np

if "/opt/trn_rl_repo" not in sys.path:
    sys.path.insert(0, "/opt/trn_rl_repo")

from concourse import bacc, mybir, tile  # noqa: E402
import concourse.bass as bass  # noqa: E402
from concourse import bass_utils  # noqa: E402

# artifact upload needs a fileshare that isn't reachable here; profiling works
# without it.
bass_utils.upload_artifacts = lambda tmpdir: "local://skipped"

F32 = mybir.dt.float32
F32R = mybir.dt.float32r
AF = mybir.ActivationFunctionType
OP = mybir.AluOpType
AX = mybir.AxisListType

B, C, H, W = 4, 256, 64, 64
HW = H * W            # 4096
K = 16
NCORES = 8
NCH = HW // 128       # 32 hw-chunks of 128 positions
BN_EPS = 1e-5
RW = 33               # per-chunk lhsT width: [R(16) | R2(16) | ones(1)]
XW = 258              # per-chunk rhs width:  [x_t(256) | ones(2)]


def build():
    nc = bacc.Bacc("TRN2", target_bir_lowering=False, debug=False,
                   num_devices=NCORES)

    x0 = nc.dram_tensor("x0", [128, HW], F32R, kind="ExternalInput")
    x1 = nc.dram_tensor("x1", [128, HW], F32R, kind="ExternalInput")
    wt = nc.dram_tensor("wt", [C, K], F32R, kind="ExternalInput")
    eye = nc.dram_tensor("eye", [128, 128], F32R, kind="ExternalInput")
    cst = nc.dram_tensor("cst", [128, 3], F32R, kind="ExternalInput")
    gam = nc.dram_tensor("gam", [1, K], F32, kind="ExternalInput")
    bet = nc.dram_tensor("bet", [1, K], F32, kind="ExternalInput")
    out = nc.dram_tensor("out", [K, C], F32, kind="ExternalOutput")

    with tile.TileContext(nc) as tc:
        with tc.tile_pool(name="data", bufs=1) as data, \
             tc.tile_pool(name="work", bufs=1) as work, \
             tc.tile_pool(name="pconv", bufs=2, space="PSUM") as pconv, \
             tc.tile_pool(name="ptr", bufs=2, space="PSUM") as ptr, \
             tc.tile_pool(name="pm", bufs=1, space="PSUM") as pmp, \
             tc.tile_pool(name="psmall", bufs=2, space="PSUM") as psm, \
             tc.tile_pool(name="dram", bufs=1, space="DRAM") as dram:

            # ---------- input DMAs (big, few) ----------
            xa = [[None, None], [None, None]]   # [c][half]: (128, 2048)
            for hf in range(2):
                for c, src in enumerate((x0, x1)):
                    t = data.tile([128, 2048], F32R, tag=f"x{c}{hf}")
                    nc.sync.dma_start(t[:], src[:, hf * 2048:(hf + 1) * 2048])
                    xa[c][hf] = t

            wt0 = data.tile([128, K], F32R, tag="wt0")
            wt1 = data.tile([128, K], F32R, tag="wt1")
            nc.scalar.dma_start(wt0[:], wt[0:128, :])
            nc.scalar.dma_start(wt1[:], wt[128:256, :])
            ident = data.tile([128, 128], F32R, tag="eye")
            nc.scalar.dma_start(ident[:], eye[:])
            cstt = data.tile([128, 3], F32R, tag="cst")
            nc.scalar.dma_start(cstt[:], cst[:])
            gamr = data.tile([1, K], F32, tag="gam")
            betr = data.tile([1, K], F32, tag="bet")
            nc.scalar.dma_start(gamr[:], gam[:])
            nc.scalar.dma_start(betr[:], bet[:])
            ones_col = cstt[:, 0:1]
            halfn = cstt[0:NCORES, 2:3]   # 1/32768 = 0.5 (pair dup) / 16384

            # ---------- big tiles ----------
            xt_all = work.tile([128, NCH * XW], F32R, tag="xtall")
            rt_all = work.tile([128, NCH * RW], F32R, tag="rtall")

            def xt_sl(m):
                return xt_all[:, m * XW:(m + 1) * XW]

            def rt_sl(m):
                return rt_all[:, m * RW:(m + 1) * RW]

            # ones columns: single strided DVE copies from cst (stride-0 dup)
            xt_ones = xt_all[:].rearrange("p (m w) -> p m w", w=XW)[:, :, 256:258]
            src2 = bass.AP(tensor=cstt[:].tensor, offset=cstt[:].offset,
                           ap=[[3, 128], [0, NCH], [1, 2]])
            nc.vector.tensor_copy(xt_ones, src2)
            rt_ones = rt_all[:].rearrange("p (m w) -> p m w", w=RW)[:, :, 32:33]
            src1 = bass.AP(tensor=cstt[:].tensor, offset=cstt[:].offset,
                           ap=[[3, 128], [0, NCH], [1, 1]])
            nc.vector.tensor_copy(rt_ones, src1)

            # ---------- conv: cw[k, j] = sum_c w[k,c] x[c,j] ----------
            cw_sb = work.tile([K, HW], F32, tag="cw")
            for t in range(8):
                hf, tt = t // 4, t % 4
                pc = pconv.tile([K, 512], F32, tag="pc")
                nc.tensor.matmul(pc[:], wt0[:],
                                 xa[0][hf][:, tt * 512:(tt + 1) * 512],
                                 start=True, stop=False)
                nc.tensor.matmul(pc[:], wt1[:],
                                 xa[1][hf][:, tt * 512:(tt + 1) * 512],
                                 start=False, stop=True)
                nc.scalar.copy(cw_sb[:, t * 512:(t + 1) * 512], pc[:])

            # ---------- transpose x -> x_t chunks ----------
            for hf in range(2):
                for tt in range(4):
                    t = 4 * hf + tt
                    for c in range(2):
                        pt = ptr.tile([128, 512], F32R, tag="pt")
                        for j in range(4):
                            nc.tensor.transpose(
                                pt[:, j * 128:(j + 1) * 128],
                                xa[c][hf][:, tt * 512 + j * 128:
                                          tt * 512 + (j + 1) * 128],
                                ident[:])
                        # one strided copy per 4-chunk pack
                        dst = bass.AP(
                            tensor=xt_all[:].tensor,
                            offset=xt_all[:].offset + (4 * t) * XW + c * 128,
                            ap=[[NCH * XW, 128], [XW, 4], [1, 128]])
                        nc.vector.tensor_copy(
                            dst, pt[:].rearrange("p (j n) -> p j n", j=4))

            # ---------- deinterleave cw -> R chunks (raw) ----------
            # r[i,k] = cw_flat[i*16+k]; chunk m <- cw[m//2, 2048*(m%2):+2048]
            engs = [nc.sync, nc.scalar, nc.gpsimd]
            for i, m in enumerate([m for m in range(NCH) if m % 2 == 0] +
                                  [m for m in range(NCH) if m % 2 == 1]):
                src = cw_sb[m // 2:m // 2 + 1,
                            2048 * (m % 2):2048 * (m % 2) + 2048]
                engs[i % 3].dma_start(
                    rt_sl(m)[:, 0:16],
                    src.rearrange("o (p k) -> o p k", p=128).bitcast(F32R))

            # ---------- S2x via ACT square+accumulate ----------
            sqparts = work.tile([128, 4], F32, tag="sqparts")
            sqdump = work.tile([128, 2048], F32, tag="sqdump")
            for c in range(2):
                for hf in range(2):
                    nc.scalar.activation(
                        sqdump[:], xa[c][hf][:].bitcast(F32), AF.Square,
                        accum_out=sqparts[:, 2 * c + hf:2 * c + hf + 1])
            sqacc = work.tile([128, 2], F32, tag="sqacc")
            nc.vector.reduce_sum(
                sqacc[:], sqparts[:].rearrange("p (c h) -> p c h", h=2),
                axis=AX.X)

            r_view = rt_all[:].rearrange("p (m w) -> p m w", w=RW)[:, :, 0:16]
            r2_view = rt_all[:].rearrange("p (m w) -> p m w", w=RW)[:, :, 16:32]

            # ---------- BN partial sums (local) ----------
            nc.vector.tensor_tensor(r2_view, r_view, r_view, op=OP.mult)
            colsums = work.tile([128, 64], F32R, tag="colsums")
            with nc.allow_low_precision(reason="f32r write of f32 partial"):
                nc.vector.reduce_sum(colsums[:, 0:32], r_view, axis=AX.X)
                nc.vector.reduce_sum(colsums[:, 32:64], r2_view, axis=AX.X)
            pbn = psm.tile([1, 64], F32, tag="small")
            nc.tensor.matmul(pbn[:], ones_col, colsums[:], start=True, stop=True)
            pbn_sb = work.tile([1, 64], F32, tag="pbnsb")
            nc.scalar.copy(pbn_sb[:], pbn[:])
            # fold chunk pairs (2k, 2k+1) -> per-k sums; [sum | sumsq]
            agin_sb = work.tile([1, 32], F32R, tag="agin")
            with nc.allow_low_precision(reason="f32r write of f32 partial"):
                nc.vector.tensor_tensor(agin_sb[:, 0:16], pbn_sb[0:1, 0:32:2],
                                        pbn_sb[0:1, 1:32:2], op=OP.add)
                nc.vector.tensor_tensor(agin_sb[:, 16:32], pbn_sb[0:1, 32:64:2],
                                        pbn_sb[0:1, 33:64:2], op=OP.add)

            # ---------- AllGather of BN partials ----------
            agin = dram.tile([1, 32], F32R)
            agout = dram.tile([NCORES, 32], F32R)
            nc.gpsimd.dma_start(agin[:], agin_sb[:])
            nc.gpsimd.collective_compute(
                "AllGather", OP.bypass,
                replica_groups=[list(range(NCORES))],
                ins=[agin[:].opt()], outs=[agout[:].opt()],
            )
            ag_sb = work.tile([NCORES, 32], F32R, tag="agsb")
            nc.gpsimd.dma_start(ag_sb[:], agout[:])

            # global [mean | E2] = (1/32768) * sum over 8 ranks
            pfold = psm.tile([1, 32], F32, tag="small")
            nc.tensor.matmul(pfold[:], halfn, ag_sb[:], start=True, stop=True)
            mrow = work.tile([1, 32], F32, tag="mrow")
            nc.scalar.copy(mrow[:], pfold[:])

            # ---------- BN affine params ----------
            rows1 = work.tile([1, 32], F32, tag="rows1")   # [g | bias]
            v_sc = work.tile([1, 16], F32, tag="vsc")
            nc.vector.tensor_tensor(v_sc[:], mrow[0:1, 0:16], mrow[0:1, 0:16],
                                    op=OP.mult)
            nc.vector.tensor_tensor(v_sc[:], mrow[0:1, 16:32], v_sc[:],
                                    op=OP.subtract)
            sd = work.tile([1, 16], F32, tag="sd")
            epsb = work.tile([1, 1], F32, tag="epsb")
            nc.vector.memset(epsb[:], BN_EPS)
            nc.scalar.activation(sd[:], v_sc[:], AF.Sqrt, bias=epsb[:])
            rq = work.tile([1, 16], F32, tag="rq")
            nc.vector.reciprocal(rq[:], sd[:])
            nc.vector.tensor_tensor(rows1[:, 0:16], gamr[:], rq[:], op=OP.mult)
            tmb = work.tile([1, 16], F32, tag="tmb")
            nc.vector.tensor_tensor(tmb[:], mrow[0:1, 0:16], rows1[:, 0:16],
                                    op=OP.mult)
            nc.vector.tensor_tensor(rows1[:, 16:32], betr[:], tmb[:],
                                    op=OP.subtract)
            # per-chunk scalar planes, broadcast from DRAM bounce:
            # gbbq[:, 0:32][p, m] = g[m//2], gbbq[:, 32:64][p, m] = bias[m//2]
            r1dup = work.tile([1, 64], F32, tag="r1dup")
            nc.vector.tensor_copy(r1dup[0:1, 0:64:2], rows1[0:1, 0:32])
            nc.vector.tensor_copy(r1dup[0:1, 1:64:2], rows1[0:1, 0:32])
            r1d = dram.tile([1, 64], F32)
            nc.scalar.dma_start(r1d[:], r1dup[0:1, 0:64])
            gbbq = work.tile([128, 64], F32, tag="gbbq")
            r1ap = r1d[0:1, 0:64]
            nc.gpsimd.dma_start(gbbq[:], bass.AP(
                tensor=r1ap.tensor, offset=r1ap.offset, ap=[[0, 128], [1, 64]]))

            # ---------- R_bn = relu(g*R + bias); R2 = R_bn^2 ----------
            def dupview(col0):
                return bass.AP(tensor=gbbq[:].tensor,
                               offset=gbbq[:].offset + col0,
                               ap=[[64, 128], [1, NCH], [0, 16]])

            nc.vector.tensor_tensor(r_view, r_view, dupview(0).bitcast(F32R),
                                    op=OP.mult)
            nc.vector.tensor_tensor(r_view, r_view, dupview(32).bitcast(F32R),
                                    op=OP.add)
            nc.scalar.activation(r_view, r_view.bitcast(F32), AF.Relu)
            nc.vector.tensor_tensor(r2_view, r_view, r_view, op=OP.mult)
            colsums2 = work.tile([128, 32], F32R, tag="colsums2")
            with nc.allow_low_precision(reason="f32r write of f32 partial"):
                nc.vector.reduce_sum(colsums2[:], r_view, axis=AX.X)

            # scale row: per-k mean over hw of cw_bn = chunk-pair totals / 4096
            psc = psm.tile([1, 32], F32, tag="small")
            nc.tensor.matmul(psc[:], ones_col, colsums2[:], start=True, stop=True)
            psc_sb = work.tile([1, 32], F32, tag="pscsb")
            nc.scalar.copy(psc_sb[:], psc[:])
            rows2 = work.tile([1, 48], F32, tag="rows2")   # [scale | Sr | SR2]
            nc.vector.tensor_tensor(v_sc[:], psc_sb[0:1, 0:32:2],
                                    psc_sb[0:1, 1:32:2], op=OP.add)
            nc.vector.tensor_scalar(rows2[:, 0:16], v_sc[:], 1.0 / HW, None,
                                    op0=OP.mult)
            # scale broadcast fires early, overlapping the M-chain
            ssb = work.tile([128, 48], F32, tag="ssb")
            r2d = dram.tile([1, 48], F32)
            nc.scalar.dma_start(r2d[0:1, 0:16], rows2[0:1, 0:16])
            r2ap0 = r2d[0:1, 0:16]
            nc.gpsimd.dma_start(ssb[:, 0:16], bass.AP(
                tensor=r2ap0.tensor, offset=r2ap0.offset, ap=[[0, 128], [1, 16]]))

            # ---------- fused M-chain: [R|R2|1]^T @ [x_t|1|1] ----------
            pM = pmp.tile([RW, XW], F32, tag="pM")
            for m in range(NCH):
                nc.tensor.matmul(pM[:], rt_sl(m), xt_sl(m),
                                 start=(m == 0), stop=(m == NCH - 1))
            M_sb = work.tile([RW, XW], F32, tag="Msb")
            nc.scalar.copy(M_sb[:], pM[:])

            # Sr | SR2: exact partition->free flip straight to DRAM, then bcast
            nc.gpsimd.dma_start(r2d[0:1, 16:48], M_sb[0:32, 256:257])
            r2ap1 = r2d[0:1, 16:48]
            nc.gpsimd.dma_start(ssb[:, 16:48], bass.AP(
                tensor=r2ap1.tensor, offset=r2ap1.offset, ap=[[0, 128], [1, 32]]))

            # ---------- final stage per C-chunk, (c, k) layout ----------
            out_sb = work.tile([K, C], F32, tag="outsb")
            for cc in range(2):
                pmt = psm.tile([128, 34], F32, tag="small")
                nc.tensor.transpose(pmt[:], M_sb[0:RW, cc * 128:(cc + 1) * 128],
                                    ident[0:RW, 0:34].bitcast(F32))
                fch = work.tile([128, 34], F32, tag="fch")
                nc.vector.tensor_copy(fch[:], pmt[:])
                # sl = scale * (S2x - 2M + SR2)
                sl = work.tile([128, 16], F32, tag="sl")
                nc.vector.tensor_scalar(sl[:], fch[:, 0:16], -2.0,
                                        sqacc[:, cc:cc + 1],
                                        op0=OP.mult, op1=OP.add)
                nc.vector.tensor_tensor(sl[:], sl[:], ssb[:, 32:48], op=OP.add)
                nc.vector.tensor_tensor(sl[:], sl[:], ssb[:, 0:16], op=OP.mult)
                # softmax over k (free axis)
                mx = work.tile([128, 1], F32, tag="mx")
                nc.vector.reduce_max(mx[:], sl[:], axis=AX.X)
                nmx = work.tile([128, 1], F32, tag="nmx")
                nc.vector.tensor_scalar(nmx[:], mx[:], -1.0, None, op0=OP.mult)
                ex = work.tile([128, 16], F32, tag="ex")
                nc.scalar.activation(ex[:], sl[:], AF.Exp, bias=nmx[:])
                sm = work.tile([128, 1], F32, tag="sm")
                nc.vector.reduce_sum(sm[:], ex[:], axis=AX.X)
                rec = work.tile([128, 1], F32, tag="rec")
                nc.vector.reciprocal(rec[:], sm[:])
                nc.vector.tensor_scalar(ex[:], ex[:], rec[:], None, op0=OP.mult)
                # enc = aw * (Sx - Sr)
                u = work.tile([128, 16], F32, tag="u")
                nc.vector.tensor_scalar(u[:], ssb[:, 16:32], -1.0,
                                        fch[:, 32:33], op0=OP.mult, op1=OP.add)
                enc = work.tile([128, 16], F32R, tag="enc")
                nc.vector.tensor_tensor(enc[:], ex[:], u[:], op=OP.mult)
                pet = psm.tile([16, 128], F32R, tag="small")
                nc.tensor.transpose(pet[:], enc[:], ident[:])
                nc.scalar.copy(out_sb[:, cc * 128:(cc + 1) * 128],
                               pet[:].bitcast(F32))

            nc.sync.dma_start(out[:], out_sb[:])

    nc.finalize()
    return nc


_NC_CACHE = None


def _get_nc():
    global _NC_CACHE
    if _NC_CACHE is None:
        _NC_CACHE = build()
    return _NC_CACHE


def _make_in_maps(x, conv_w, bn_gamma, bn_beta):
    xr = np.ascontiguousarray(x.reshape(B, C, HW), dtype=np.float32)
    wtm = np.ascontiguousarray(conv_w.T, dtype=np.float32)
    eye = np.eye(128, dtype=np.float32)
    cstm = np.zeros((128, 3), dtype=np.float32)
    cstm[:, 0] = 1.0
    cstm[:, 1] = 1.0
    cstm[:, 2] = 1.0 / 32768.0
    gamv = np.ascontiguousarray(bn_gamma.reshape(1, K), dtype=np.float32)
    betv = np.ascontiguousarray(bn_beta.reshape(1, K), dtype=np.float32)
    in_maps = []
    for p in range(NCORES):
        b = p // 2
        in_maps.append({
            "x0": np.ascontiguousarray(xr[b, 0:128, :]),
            "x1": np.ascontiguousarray(xr[b, 128:256, :]),
            "wt": wtm, "eye": eye, "cst": cstm,
            "gam": gamv, "bet": betv,
        })
    return in_maps


def run(x, conv_w, bn_gamma, bn_beta, trace=False):
    nc = _get_nc()
    in_maps = _make_in_maps(x, conv_w, bn_gamma, bn_beta)
    res = bass_utils.run_bass_kernel_spmd(
        nc, in_maps, core_ids=list(range(NCORES)), trace=trace)
    outp = np.stack([res.results[2 * b]["out"]
                     for b in range(B)]).astype(np.float32)
    return outp, res


def kernel(x, conv_w, bn_gamma, bn_beta):
    outp, _ = run(np.asarray(x), np.asarray(conv_w),
                  np.asarray(bn_gamma), np.asarray(bn_beta))
    return outp
